# revision 2
# baseline (speedup 1.0000x reference)
"""Causal multi-head attention (RoPE) TRN2 Bass kernel.

Problem: x[2,2048,2048] fp32, Wq/Wk/Wv/Wo [2048,2048], 16 heads, d_k=128,
causal softmax attention with interleaved RoPE, out = attn_out @ Wo.

Sharding (8 cores): core = b*4 + g handles batch b and head group g
(4 heads = 512 feature columns). Wq/Wk/Wv split column-wise, Wo row-wise;
the "all-reduce" after the output projection is done on the host by summing
the 4 partial outputs per batch (gather/unshard step).

Device kernel (per core), all matmuls in float32r (fp32 storage, reduced
multiplier precision, full PE rate at free-dim >= 256):
  phase A (per 512-row chunk j): QT/KT = (x @ Wq/Wk)^T via lhsT=W tiles,
    rhs=xT; RoPE applied in [d_k, s] layout using a host-side permutation of
    W columns into half-split order (rotate-half form, sign baked into sinT).
    V = x @ Wv in natural [s, feat] layout via lhsT=xT tiles.
  phase B: causal attention for q-tile j, scores computed transposed
    (S^T[k,q]) so softmax-normalized weights feed attn@V without transposes;
    exp on ScalarE (no max subtraction needed: scores are O(5)), row sums via
    an all-ones lhsT matmul (broadcast over partitions), division deferred to
    the [d_k, s] output tile.
  phase C: out_partial[chunk rows] = O @ Wo via lhsT=OT tiles.

RoPE pair trick: scores are invariant under any permutation of d_k applied
to both Q and K, so W columns are permuted per head to [even..., odd...] on
the host; the rotate pairs then live 64 partitions apart (two plain
partition-offset copies instead of an interleaved shuffle), and cosT/sinT
are permuted/sign-baked to match.
"""

import math
import sys

sys.path.insert(0, "/opt/trn_rl_repo")

import numpy as np

D_MODEL = 2048
SEQ = 2048
BATCH = 2
N_CORES = 8
HEADS_PER_CORE = 4
GCOLS = HEADS_PER_CORE * 128  # 512 feature columns per core
KB = D_MODEL // 128  # 16 contraction blocks
N_CHUNKS = SEQ // 512  # 4
SCALE = 1.0 / math.sqrt(128.0)

_CACHE = {}


def _build_program():
    import concourse.mybir as mybir
    import concourse.tile as tile
    from concourse import bacc

    F = mybir.dt.float32
    FR = mybir.dt.float32r
    AF = mybir.ActivationFunctionType

    nc = bacc.Bacc("TRN2", target_bir_lowering=False, debug=False,
                   num_devices=N_CORES)

    xT_d = nc.dram_tensor("xT", (D_MODEL, SEQ), FR, kind="ExternalInput").ap()
    Wq_d = nc.dram_tensor("Wq", (D_MODEL, GCOLS), FR, kind="ExternalInput").ap()
    Wk_d = nc.dram_tensor("Wk", (D_MODEL, GCOLS), FR, kind="ExternalInput").ap()
    Wv_d = nc.dram_tensor("Wv", (D_MODEL, GCOLS), FR, kind="ExternalInput").ap()
    Wo_d = nc.dram_tensor("Wo", (GCOLS, D_MODEL), FR, kind="ExternalInput").ap()
    cosT_d = nc.dram_tensor("cosT", (128, SEQ), F, kind="ExternalInput").ap()
    sinT_d = nc.dram_tensor("sinT", (128, SEQ), F, kind="ExternalInput").ap()
    mask_d = nc.dram_tensor("mask", (128, 896), FR, kind="ExternalInput").ap()
    out_d = nc.dram_tensor("out", (SEQ, D_MODEL), F, kind="ExternalOutput").ap()

    with tile.TileContext(nc) as tc:
        with tc.tile_pool(name="resid", bufs=1) as resid, \
             tc.tile_pool(name="xtp", bufs=1) as xtp, \
             tc.tile_pool(name="wqkp", bufs=2) as wqkp, \
             tc.tile_pool(name="wvp", bufs=2) as wvp, \
             tc.tile_pool(name="csp", bufs=2) as csp, \
             tc.tile_pool(name="qtp", bufs=1) as qtp, \
             tc.tile_pool(name="otp", bufs=1) as otp, \
             tc.tile_pool(name="ep", bufs=3) as ep, \
             tc.tile_pool(name="ropep", bufs=2) as ropep, \
             tc.tile_pool(name="rcp", bufs=2) as rcp, \
             tc.tile_pool(name="outp", bufs=3) as outp, \
             tc.tile_pool(name="psA", bufs=5, space="PSUM") as psA, \
             tc.tile_pool(name="psB", bufs=3, space="PSUM") as psB:

            ones_f = resid.tile([128, 128], F, tag="ones_f")
            nc.vector.memset(ones_f[:], 1.0)
            ones = resid.tile([128, 128], FR, tag="ones")
            nc.vector.tensor_copy(ones[:], ones_f[:])
            mask_sb = resid.tile([128, 896], FR, tag="mask")
            nc.sync.dma_start(mask_sb[:], mask_d[:])
            KT = resid.tile([128, HEADS_PER_CORE, SEQ], FR, tag="KT")
            V = resid.tile([128, KB, GCOLS], FR, tag="V")
            wo = resid.tile([128, HEADS_PER_CORE, D_MODEL], FR, tag="wo")
            nc.sync.dma_start(wo[:], Wo_d.rearrange("(c p) n -> p c n", p=128))

            for j in range(N_CHUNKS):
                ssl = slice(j * 512, (j + 1) * 512)

                xt = xtp.tile([128, KB, 512], FR, tag="xt")
                nc.sync.dma_start(
                    xt[:], xT_d[:, ssl].rearrange("(ko p) s -> p ko s", p=128))
                cos_t = csp.tile([128, 512], F, tag="cos")
                nc.sync.dma_start(cos_t[:], cosT_d[:, ssl])
                sin_t = csp.tile([128, 512], F, tag="sin")
                nc.sync.dma_start(sin_t[:], sinT_d[:, ssl])
                qt = qtp.tile([128, HEADS_PER_CORE, 512], FR, tag="qt")

                # --- Q/K projections + RoPE (outputs transposed: [d_k, s]) ---
                for dst, is_q, W_d in ((qt, True, Wq_d), (KT, False, Wk_d)):
                    for m in range(HEADS_PER_CORE):
                        w = wqkp.tile([128, KB, 128], FR, tag="wqk")
                        nc.sync.dma_start(
                            w[:],
                            W_d[:, m * 128:(m + 1) * 128].rearrange(
                                "(ko p) m -> p ko m", p=128))
                        ps = psA.tile([128, 512], F, tag="flow")
                        for k in range(KB):
                            nc.tensor.matmul(ps[:], w[:, k], xt[:, k],
                                             start=(k == 0), stop=(k == KB - 1))
                        rot = ropep.tile([128, 512], F, tag="rot")
                        nc.vector.tensor_copy(rot[:64, :], ps[64:128, :])
                        nc.vector.tensor_copy(rot[64:128, :], ps[:64, :])
                        tmp = ropep.tile([128, 512], F, tag="tmp")
                        nc.vector.tensor_mul(tmp[:], ps[:], cos_t[:])
                        nc.vector.tensor_mul(rot[:], rot[:], sin_t[:])
                        out_ap = dst[:, m, :] if is_q else dst[:, m, ssl]
                        nc.vector.tensor_add(out_ap, tmp[:], rot[:])

                # --- V projection (natural layout [s, feat]) ---
                vps = [psA.tile([128, 512], F, tag="flow", name=f"vps{m}")
                       for m in range(4)]
                for k in range(KB):
                    wv = wvp.tile([128, 512], FR, tag="wv")
                    nc.sync.dma_start(wv[:], Wv_d[k * 128:(k + 1) * 128, :])
                    for m in range(4):
                        nc.tensor.matmul(
                            vps[m][:], xt[:, k, m * 128:(m + 1) * 128], wv[:],
                            start=(k == 0), stop=(k == KB - 1))
                for m in range(4):
                    nc.any.tensor_copy(V[:, 4 * j + m, :], vps[m][:])

                # --- causal attention for q-tile j ---
                ot = otp.tile([128, HEADS_PER_CORE, 512], FR, tag="ot")
                last = 4 * j + 3
                for h in range(HEADS_PER_CORE):
                    rs_ps = psB.tile([128, 512], F, tag="hold")
                    o_ps = psB.tile([128, 512], F, tag="hold")
                    prev_e = None
                    prev_kb = -1
                    for kb in range(4 * j + 4):
                        s_ps = psA.tile([128, 512], F, tag="flow")
                        nc.tensor.matmul(
                            s_ps[:], KT[:, h, kb * 128:(kb + 1) * 128],
                            qt[:, h, :], start=True, stop=True)
                        if prev_e is not None:
                            nc.tensor.matmul(rs_ps[:], ones[:], prev_e[:],
                                             start=(prev_kb == 0), stop=False)
                            nc.tensor.matmul(
                                o_ps[:],
                                V[:, prev_kb, h * 128:(h + 1) * 128],
                                prev_e[:], start=(prev_kb == 0), stop=False)
                        e = ep.tile([128, 512], FR, tag="e")
                        nc.scalar.activation(e[:], s_ps[:], AF.Exp, scale=SCALE)
                        d = kb - 4 * j
                        if d >= 0:
                            nc.vector.tensor_mul(
                                e[:], e[:],
                                mask_sb[:, 384 - 128 * d: 896 - 128 * d])
                        prev_e = e
                        prev_kb = kb
                    nc.tensor.matmul(rs_ps[:], ones[:], prev_e[:],
                                     start=(prev_kb == 0), stop=True)
                    nc.tensor.matmul(
                        o_ps[:], V[:, prev_kb, h * 128:(h + 1) * 128],
                        prev_e[:], start=(prev_kb == 0), stop=True)
                    rc = rcp.tile([128, 512], F, tag="rc")
                    nc.vector.reciprocal(rc[:], rs_ps[:])
                    nc.vector.tensor_mul(ot[:, h, :], o_ps[:], rc[:])

                # --- output projection for chunk rows ---
                for m in range(4):
                    for n in range(4):
                        ps = psA.tile([128, 512], F, tag="flow")
                        for c in range(HEADS_PER_CORE):
                            nc.tensor.matmul(
                                ps[:], ot[:, c, m * 128:(m + 1) * 128],
                                wo[:, c, n * 512:(n + 1) * 512],
                                start=(c == 0), stop=(c == 3))
                        ob = outp.tile([128, 512], F, tag="ob")
                        nc.any.tensor_copy(ob[:], ps[:])
                        nc.sync.dma_start(
                            out_d[(4 * j + m) * 128:(4 * j + m + 1) * 128,
                                  n * 512:(n + 1) * 512], ob[:])

    nc.compile()
    return nc


def _get_program():
    if "nc" not in _CACHE:
        _CACHE["nc"] = _build_program()
    return _CACHE["nc"]


def _host_prep(x, token_positions, Wq, Wk, Wv, Wo):
    x = np.asarray(x, dtype=np.float32)
    Wq = np.asarray(Wq, dtype=np.float32)
    Wk = np.asarray(Wk, dtype=np.float32)
    Wv = np.asarray(Wv, dtype=np.float32)
    Wo = np.asarray(Wo, dtype=np.float32)
    pos = np.asarray(token_positions).astype(np.float64)

    # RoPE tables in permuted (half-split) layout, transposed to [d_k, s].
    inv = 10000.0 ** (-2.0 * np.arange(64, dtype=np.float64) / 128.0)
    ang = inv[:, None] * pos[None, :]  # [64, S]
    cos_h = np.cos(ang)
    sin_h = np.sin(ang)
    cosT = np.concatenate([cos_h, cos_h], axis=0).astype(np.float32)
    sinT = np.concatenate([-sin_h, sin_h], axis=0).astype(np.float32)

    # half-split permutation of each head's 128 feature columns
    perm = np.concatenate([np.arange(0, 128, 2), np.arange(1, 128, 2)])

    # causal mask strip: mask[p, g] = 1 iff p <= g - 384; diagonal block d
    # (d = kb - 4j) uses columns [384-128d, 896-128d).
    mask = (np.arange(128)[:, None] <= np.arange(896)[None, :] - 384)
    mask = np.ascontiguousarray(mask.astype(np.float32))

    def permute_cols(W):  # [2048, 512] -> per-head column permutation
        return np.ascontiguousarray(
            W.reshape(D_MODEL, HEADS_PER_CORE, 128)[:, :, perm].reshape(
                D_MODEL, GCOLS))

    in_maps = []
    for core in range(N_CORES):
        b, g = divmod(core, 4)
        cols = slice(g * GCOLS, (g + 1) * GCOLS)
        in_maps.append({
            "xT": np.ascontiguousarray(x[b].T),
            "Wq": permute_cols(Wq[:, cols]),
            "Wk": permute_cols(Wk[:, cols]),
            "Wv": np.ascontiguousarray(Wv[:, cols]),
            "Wo": np.ascontiguousarray(Wo[cols, :]),
            "cosT": cosT,
            "sinT": sinT,
            "mask": mask,
        })
    return in_maps


def run_sharded(x, token_positions, Wq, Wk, Wv, Wo, trace=False, tmpdir=None):
    """Run the SPMD kernel; returns (full_output, BassKernelResults)."""
    from concourse import bass_utils

    nc = _get_program()
    in_maps = _host_prep(x, token_positions, Wq, Wk, Wv, Wo)
    kwargs = {}
    if trace:
        kwargs = {"trace": True, "tmpdir": tmpdir}
    res = bass_utils.run_bass_kernel_spmd(
        nc, in_maps, core_ids=list(range(N_CORES)), **kwargs)
    out = np.empty((BATCH, SEQ, D_MODEL), dtype=np.float32)
    for b in range(BATCH):
        acc = np.zeros((SEQ, D_MODEL), dtype=np.float64)
        for g in range(4):
            acc += res.results[b * 4 + g]["out"]
        out[b] = acc.astype(np.float32)
    return out, res


def kernel(x, token_positions, Wq, Wk, Wv, Wo):
    out, _ = run_sharded(x, token_positions, Wq, Wk, Wv, Wo)
    return out
